# revision 32
# baseline (speedup 1.0000x reference)
"""Trainium2 Bass kernel for nn_EnhancedFreqLCBlock.

Self-contained: accepts FULL inputs, returns FULL output.
Sharding: 8 cores = 2 batches x 4 quadrant Mamba blocks (expert parallel).
Per core: mask -> quadrant 2D-DCT (fp16 matmuls) -> channel LN -> Mamba
(bf16 expansions on PE, exp on ACT, scan on DVE in 512-chunks, ch-mult on
GpSimd) -> residual -> quadrant IDCT contribution. Host sums quadrant
contributions per batch.
"""
import numpy as np

B, C, H, W = 2, 96, 128, 128
HQ, WQ = H // 2, W // 2          # 64, 64
L = HQ * WQ                      # 4096
D = 192                          # d_inner
S = 16                           # d_state
RK = 6                           # dt_rank
KCONV = 4
NCHUNK = 8
LC = L // NCHUNK                 # 512
NT = (D * S) // 128              # 24 scan partition-tiles
DA, DB = 128, 64                 # d split 192 = 128 + 64

_BUILT = {}


def _dct_mat(N):
    n = np.arange(N)
    M = np.cos(np.pi * (2 * n[None, :] + 1) * n[:, None] / (2 * N)) * np.sqrt(2.0 / N)
    M[0] *= 1.0 / np.sqrt(2.0)
    return M.astype(np.float32)


def _build_nc():
    import concourse.bacc as bacc
    import concourse.bass as bass
    import concourse.mybir as mybir
    import concourse.tile as tile

    f32 = mybir.dt.float32
    f16 = mybir.dt.float16
    bf16 = mybir.dt.bfloat16
    AF = mybir.ActivationFunctionType
    OP = mybir.AluOpType
    AX = mybir.AxisListType
    ts = bass.ts

    nc = bacc.Bacc()

    # ---------------- DRAM I/O ----------------
    xb = nc.dram_tensor("xb", [C, H, W], f32, kind="ExternalInput")
    d_mhqT = nc.dram_tensor("mhqT", [H, HQ], f16, kind="ExternalInput")
    d_mwqT = nc.dram_tensor("mwqT", [W, WQ], f16, kind="ExternalInput")
    d_mhq = nc.dram_tensor("mhq", [HQ, H], f16, kind="ExternalInput")
    d_mwq = nc.dram_tensor("mwq", [WQ, W], f16, kind="ExternalInput")
    d_identh = nc.dram_tensor("identh", [WQ, WQ], f16, kind="ExternalInput")
    d_identb = nc.dram_tensor("identb", [C, C], bf16, kind="ExternalInput")
    d_inwT = nc.dram_tensor("inwT", [C, 2 * D], bf16, kind="ExternalInput")
    d_biasi = nc.dram_tensor("biasi", [D, 1], f32, kind="ExternalInput")
    d_biasz = nc.dram_tensor("biasz", [D, 1], f32, kind="ExternalInput")
    d_convw = nc.dram_tensor("convw", [D, KCONV], f32, kind="ExternalInput")
    d_convb = nc.dram_tensor("convb", [D, 1], f32, kind="ExternalInput")
    d_xpwT = nc.dram_tensor("xpwT", [D, 96], bf16, kind="ExternalInput")
    d_dtwT = nc.dram_tensor("dtwT", [RK, D], bf16, kind="ExternalInput")
    d_dtb = nc.dram_tensor("dtb", [D, 1], f32, kind="ExternalInput")
    d_acol = nc.dram_tensor("acol", [128, NT], f32, kind="ExternalInput")
    d_dp = nc.dram_tensor("dp", [D, 1], f32, kind="ExternalInput")
    d_outwT = nc.dram_tensor("outwT", [D, C], bf16, kind="ExternalInput")
    d_p01all = nc.dram_tensor("p01all", [128, 128 * 16], bf16, kind="ExternalInput")
    d_p01ball = nc.dram_tensor("p01ball", [64, 128 * 8], bf16, kind="ExternalInput")
    d_s01 = nc.dram_tensor("s01", [S, 128], bf16, kind="ExternalInput")
    d_r01all = nc.dram_tensor("r01all", [128, 128 * 16], bf16, kind="ExternalInput")
    d_r01ball = nc.dram_tensor("r01ball", [128, 64 * 8], bf16, kind="ExternalInput")
    contrib = nc.dram_tensor("contrib", [C, H, W], f32, kind="ExternalOutput")

    with tile.TileContext(nc) as tc:
        consts = tc.alloc_tile_pool(name="consts", bufs=1)

        def cload(dram, shape, dt=f32):
            t = consts.tile(shape, dt, name=f"c_{dram.name}")
            nc.sync.dma_start(t[:], dram[:])
            return t

        def cload2(dram, dt=f32):
            ta = consts.tile([DA] + list(dram.shape[1:]), dt, name=f"cA_{dram.name}")
            nc.sync.dma_start(ta[:], dram[0:DA])
            tb = consts.tile([DB] + list(dram.shape[1:]), dt, name=f"cB_{dram.name}")
            nc.sync.dma_start(tb[:], dram[DA:D])
            return ta, tb

        mhqT = cload(d_mhqT, [H, HQ], f16)
        mwqT = cload(d_mwqT, [W, WQ], f16)
        mhq = cload(d_mhq, [HQ, H], f16)
        mwq = cload(d_mwq, [WQ, W], f16)
        identh = cload(d_identh, [WQ, WQ], f16)
        identb = cload(d_identb, [C, C], bf16)
        inwT = cload(d_inwT, [C, 2 * D], bf16)
        biasiA, biasiB = cload2(d_biasi)
        biaszA, biaszB = cload2(d_biasz)
        convwA, convwB = cload2(d_convw)
        convbA, convbB = cload2(d_convb)
        xpwTA, xpwTB = cload2(d_xpwT, bf16)
        dtwT = cload(d_dtwT, [RK, D], bf16)
        dtbA, dtbB = cload2(d_dtb)
        acol = cload(d_acol, [128, NT])
        dpA, dpB = cload2(d_dp)
        outwTA, outwTB = cload2(d_outwT, bf16)
        p01all = cload(d_p01all, [128, 128 * 16], bf16)
        p01ball = cload(d_p01ball, [64, 128 * 8], bf16)
        s01 = cload(d_s01, [S, 128], bf16)
        r01all = cload(d_r01all, [128, 128 * 16], bf16)
        r01ball = cload(d_r01ball, [128, 64 * 8], bf16)
        ones96 = consts.tile([C, 1], f16)
        nc.vector.memset(ones96[:], 1.0)
        onesr = consts.tile([1, 128], f32)
        nc.vector.memset(onesr[:], 1.0)
        eps64 = consts.tile([WQ, 1], f32)
        nc.vector.memset(eps64[:], 1e-5)

        # psum pools: one unified ring of [<=128, <=512]-f32 slots + yps
        pr = tc.alloc_tile_pool(name="pr", bufs=5, space="PSUM")
        py = tc.alloc_tile_pool(name="py", bufs=1, space="PSUM")   # yps 3 banks

        def ttile(p, n, nm, dt=f32):
            return pr.tile([p, n], dt, name=nm, tag="rr")

        jtile = ttile

        # =============== Phase A: load + mask ===============
        pD_ = tc.alloc_tile_pool(name="pD", bufs=1)
        pB = tc.alloc_tile_pool(name="pB", bufs=1)
        pXH = tc.alloc_tile_pool(name="pXH", bufs=1)
        pA = tc.alloc_tile_pool(name="pA", bufs=1)
        xc = pA.tile([C, H * W], f32)
        xb_c = xb.rearrange("c h w -> c (h w)")
        for k in (2, 0, 1, 3):
            nc.sync.dma_start(xc[:, ts(k, H * W // 4)], xb_c[:, ts(k, H * W // 4)])
        xh = pXH.tile([H, C * W], f32)
        xb_h = xb.rearrange("c h w -> h c w")
        xh3 = xh.rearrange("h (c w) -> h c w", c=C)
        dma_engs = [nc.scalar, nc.gpsimd, nc.scalar, nc.gpsimd]
        for i in range(4):
            dma_engs[i].dma_start(xh3[ts(i, 32), :, :], xb_h[ts(i, 32), :, :])

        cpos = (H // 2) * W + (W // 2)
        center = xc[:, cpos:cpos + 1]

        # ||center||^2 -> 0.49*||c||^2 broadcast to 128 partitions
        cn_ps = ttile(1, 1, "cn_ps")
        nc.tensor.matmul(cn_ps[:], center, center, start=True, stop=True)
        s049 = pA.tile([1, 1], f32)
        nc.vector.tensor_scalar_mul(s049[:], cn_ps[:], 0.49)
        s049p = ttile(128, 1, "s049p")
        nc.tensor.matmul(s049p[:], onesr[:], s049[:], start=True, stop=True)
        s049b = pA.tile([128, 1], f32)
        nc.vector.tensor_copy(s049b[:], s049p[:])

        # num matmuls (fp32, PE idle during loads) + full-partition ssq chunks
        num_hw = pA.tile([128, 128], f32)
        ssq_hw = pA.tile([128, 128], f32)
        thr = pA.tile([128, 128], f32)
        mask_hw = pA.tile([128, 128], f32)
        xhm = pXH.tile([H, C * W], f16)
        xhm3 = xhm.rearrange("h (c w) -> h c w", c=C)
        pSt = tc.alloc_tile_pool(name="pSt", bufs=3)
        for i in range(32):
            nps = ttile(1, LC, "nps")
            nc.tensor.matmul(nps[:], center, xc[:, ts(i, LC)],
                             start=True, stop=True)
            nrow = pSt.tile([1, LC], f32, name="nrow", bufs=2)
            nc.scalar.activation(nrow[:], nps[:], AF.Copy)
            nc.sync.dma_start(num_hw[ts(i, 4), :], nrow[:])
        for wb in range(4):
            xhsq = pSt.tile([H, C * 32], f16, name="xhsq", bufs=2)
            xhsq3 = xhsq.rearrange("h (c w) -> h c w", c=C)
            nc.scalar.activation(xhsq3[:, :, :], xh3[:, :, ts(wb, 32)], AF.Square)
            nc.vector.tensor_reduce(
                ssq_hw[:, ts(wb, 32)], xhsq3.transpose([0, 2, 1]),
                axis=AX.X, op=OP.add)
        nc.scalar.activation(thr[:], ssq_hw[:], AF.Sqrt, bias=0.0, scale=s049b[:])
        nc.vector.tensor_scalar_add(thr[:], thr[:], 0.7e-6)
        nc.vector.tensor_tensor(mask_hw[:], num_hw[:], thr[:], op=OP.is_ge)
        # apply per-c-block so DCT stage 1 can start on early c blocks
        for cb in range(4):
            nc.vector.tensor_tensor(
                xhm3[:, ts(cb, 24), :], xh3[:, ts(cb, 24), :],
                mask_hw[:, None, :].broadcast_to([128, 24, 128]), op=OP.mult)
        pSt.release()
        pA.release()

        # =============== Phase B: forward DCT (f16) ===============
        # t2[w, c, hq] = sum_h x[h, c, w] * MhqT[h, hq]; 8 c per psum group
        t2 = pB.tile([W, C * HQ], f16)
        t2_3 = t2.rearrange("p (c q) -> p c q", c=C)
        for g in range(12):
            tps = ttile(W, 8 * HQ, "tps")
            tps3 = tps.rearrange("p (c q) -> p c q", c=8)
            for cc in range(8):
                c = 8 * g + cc
                nc.tensor.matmul(tps3[:, cc, :], xhm3[:, c, :], mhqT[:],
                                 start=True, stop=True)
            nc.scalar.activation(t2_3[:, ts(g, 8), :], tps3[:, :, :], AF.Copy)
        pXH.release()

        # xdq[wq, c, hq] = sum_w MwqT[w, wq] t2[w, (c hq)]
        xdq = pD_.tile([WQ, C * HQ], f16)
        xdq3 = xdq.rearrange("p (c q) -> p c q", c=C)
        for i in range(12):
            xps = ttile(WQ, LC, "xps")
            nc.tensor.matmul(xps[:], mwqT[:], t2[:, ts(i, LC)], start=True, stop=True)
            nc.scalar.activation(xdq[:, ts(i, LC)], xps[:], AF.Copy)
        pB.release()

        # =============== Phase C: LayerNorm over c ===============
        pG = tc.alloc_tile_pool(name="pG", bufs=1)
        pF = tc.alloc_tile_pool(name="pF", bufs=1)
        pE = tc.alloc_tile_pool(name="pE", bufs=1)
        pC = tc.alloc_tile_pool(name="pC", bufs=1)
        xdq_whc = xdq3.transpose([0, 2, 1])  # [wq, hq, c] view
        smu = pC.tile([WQ, HQ], f32)
        nc.vector.tensor_reduce(smu[:], xdq_whc, axis=AX.X, op=OP.add)
        xdsq = pC.tile([WQ, C * HQ], f16)
        nc.scalar.activation(xdsq[:], xdq[:], AF.Square)
        ssq2 = pC.tile([WQ, HQ], f32)
        nc.vector.tensor_reduce(
            ssq2[:], xdsq.rearrange("p (c q) -> p c q", c=C).transpose([0, 2, 1]),
            axis=AX.X, op=OP.add)
        mu = pC.tile([WQ, HQ], f32)
        nc.vector.tensor_scalar_mul(mu[:], smu[:], 1.0 / C)
        var = pC.tile([WQ, HQ], f32)
        nc.vector.tensor_scalar_mul(ssq2[:], ssq2[:], 1.0 / C)
        nc.vector.tensor_tensor(var[:], mu[:], mu[:], op=OP.mult)
        nc.vector.tensor_tensor(var[:], ssq2[:], var[:], op=OP.subtract)
        sd = pC.tile([WQ, HQ], f32)
        nc.scalar.activation(sd[:], var[:], AF.Sqrt, bias=eps64[:])
        inv = pC.tile([WQ, HQ], f32)
        nc.vector.reciprocal(inv[:], sd[:])
        mu16 = pC.tile([WQ, HQ], f16)
        nc.vector.tensor_copy(mu16[:], mu[:])
        inv16 = pC.tile([WQ, HQ], f16)
        nc.vector.tensor_copy(inv16[:], inv[:])
        xn = pC.tile([WQ, C * HQ], f16)
        xn3 = xn.rearrange("p (c q) -> p c q", c=C)
        nc.vector.tensor_tensor(
            xn3[:, :, :], xdq3[:, :, :],
            mu16[:, None, :].broadcast_to([WQ, C, HQ]), op=OP.subtract)
        nc.vector.tensor_tensor(
            xn3[:, :, :], xn3[:, :, :],
            inv16[:, None, :].broadcast_to([WQ, C, HQ]), op=OP.mult)
        xn_c = pE.tile([C, L], bf16)
        for g in range(8):
            tp2 = ttile(C, 8 * WQ, "tp2", f16)
            for k in range(8):
                hq = 8 * g + k
                nc.tensor.matmul(tp2[:, ts(k, WQ)], xn3[:, :, hq], identh[:],
                                 is_transpose=True, start=True, stop=True)
            nc.scalar.activation(xn_c[:, ts(g, 8 * WQ)], tp2[:], AF.Copy)
        pC.release()

        # =============== Phase D: in_proj + conv + silu ===============
        # xi: padded conv input [D, 3 + L] bf16; sz = silu(z)
        xiA = pF.tile([DA, KCONV - 1 + L], bf16)
        xiB = pF.tile([DB, KCONV - 1 + L], bf16)
        nc.vector.memset(xiA[:, 0:KCONV - 1], 0.0)
        nc.vector.memset(xiB[:, 0:KCONV - 1], 0.0)
        szA = pG.tile([DA, L], bf16)
        szB = pG.tile([DB, L], bf16)
        for i in range(NCHUNK):
            ps0 = ttile(128, LC, "ps0")
            nc.tensor.matmul(ps0[:], inwT[:, 0:128], xn_c[:, ts(i, LC)],
                             start=True, stop=True)
            ps1 = ttile(128, LC, "ps1")
            nc.tensor.matmul(ps1[:], inwT[:, 128:256], xn_c[:, ts(i, LC)],
                             start=True, stop=True)
            ps2 = ttile(128, LC, "ps2")
            nc.tensor.matmul(ps2[:], inwT[:, 256:384], xn_c[:, ts(i, LC)],
                             start=True, stop=True)
            o = KCONV - 1 + i * LC
            nc.scalar.activation(xiA[:, o:o + LC], ps0[:], AF.Identity,
                                 bias=biasiA[:])
            nc.scalar.activation(xiB[:, o:o + LC], ps1[0:64, :], AF.Identity,
                                 bias=biasiB[:])
            nc.scalar.activation(szA[0:64, ts(i, LC)], ps1[64:128, :], AF.Silu,
                                 bias=biaszA[0:64, :])
            nc.scalar.activation(szA[64:128, ts(i, LC)], ps2[0:64, :], AF.Silu,
                                 bias=biaszA[64:128, :])
            nc.scalar.activation(szB[:, ts(i, LC)], ps2[64:128, :], AF.Silu,
                                 bias=biaszB[:])
        pE.release()

        # conv: 4-tap via DVE tensor_scalar/STT, then ACT silu
        xi2A = pG.tile([DA, L], bf16)
        xi2B = pG.tile([DB, L], bf16)
        pCv = tc.alloc_tile_pool(name="pCv", bufs=2)
        for i in range(NCHUNK):
            cvA = pCv.tile([DA, LC], bf16, name="cvA")
            cvB = pCv.tile([DB, LC], bf16, name="cvB")
            nc.vector.tensor_scalar_mul(cvA[:], xiA[:, i * LC:(i + 1) * LC],
                                        convwA[:, 0:1])
            nc.vector.tensor_scalar_mul(cvB[:], xiB[:, i * LC:(i + 1) * LC],
                                        convwB[:, 0:1])
            for k in range(1, KCONV):
                nc.vector.scalar_tensor_tensor(
                    cvA[:], xiA[:, k + i * LC:k + (i + 1) * LC],
                    convwA[:, k:k + 1], cvA[:], op0=OP.mult, op1=OP.add)
                nc.vector.scalar_tensor_tensor(
                    cvB[:], xiB[:, k + i * LC:k + (i + 1) * LC],
                    convwB[:, k:k + 1], cvB[:], op0=OP.mult, op1=OP.add)
            nc.scalar.activation(xi2A[:, ts(i, LC)], cvA[:], AF.Silu, bias=convbA[:])
            nc.scalar.activation(xi2B[:, ts(i, LC)], cvB[:], AF.Silu, bias=convbB[:])
        pCv.release()
        pF.release()

        # ====== Phase F: scan loop, 4 x LCF=1024 ======
        pT = tc.alloc_tile_pool(name="pT", bufs=2)
        LCF = 2 * LC
        # full-L prep: x_proj, delta (softplus), dX, B/C expansions
        deltaAf = pG.tile([DA, L], bf16)
        deltaBf = pG.tile([DB, L], bf16)
        dXAf = pG.tile([DA, L], bf16)
        dXBf = pG.tile([DB, L], bf16)
        brepf = pG.tile([128, L], bf16)
        crepf = pG.tile([128, L], bf16)
        hlast = pG.tile([128, NT], bf16)
        pP = tc.alloc_tile_pool(name="pP", bufs=2)
        for k in range(NCHUNK):
            sl = ts(k, LC)
            xpps = ttile(96, LC, "xpps")
            nc.tensor.matmul(xpps[:], xpwTA[:], xi2A[:, sl],
                             start=True, stop=False)
            nc.tensor.matmul(xpps[:], xpwTB[:], xi2B[:, sl],
                             start=False, stop=True)
            dtt = pP.tile([RK, LC], bf16, name="dtt")
            nc.vector.tensor_copy(dtt[:], xpps[0:RK, :])
            bmt = pP.tile([S, LC], bf16, name="bmt")
            nc.vector.tensor_copy(bmt[:], xpps[32:32 + S, :])
            cmt = pP.tile([S, LC], bf16, name="cmt")
            nc.vector.tensor_copy(cmt[:], xpps[64:64 + S, :])
            dtpA = ttile(DA, LC, "dtpA")
            nc.tensor.matmul(dtpA[:], dtwT[:, 0:DA], dtt[:],
                             start=True, stop=True)
            nc.scalar.activation(deltaAf[:, sl], dtpA[:], AF.Exp, bias=dtbA[:])
            dtpB = ttile(DB, LC, "dtpB")
            nc.tensor.matmul(dtpB[:], dtwT[:, DA:D], dtt[:],
                             start=True, stop=True)
            nc.scalar.activation(deltaBf[:, sl], dtpB[:], AF.Exp, bias=dtbB[:])
            brps = ttile(128, LC, "brps")
            nc.tensor.matmul(brps[:], s01[:], bmt[:],
                             start=True, stop=True)
            nc.scalar.activation(brepf[:, sl], brps[:], AF.Copy)
            crps = ttile(128, LC, "crps")
            nc.tensor.matmul(crps[:], s01[:], cmt[:],
                             start=True, stop=True)
            nc.scalar.activation(crepf[:, sl], crps[:], AF.Copy)
            if k == 1:
                fs = bass.ds(0, LCF)
                nc.scalar.activation(deltaAf[:, fs], deltaAf[:, fs],
                                     AF.Ln, bias=1.0)
                nc.scalar.activation(deltaBf[:, fs], deltaBf[:, fs],
                                     AF.Ln, bias=1.0)
                nc.vector.tensor_tensor(dXAf[:, fs], deltaAf[:, fs],
                                        xi2A[:, fs], op=OP.mult)
                nc.vector.tensor_tensor(dXBf[:, fs], deltaBf[:, fs],
                                        xi2B[:, fs], op=OP.mult)
            elif k == NCHUNK - 1:
                fs = bass.ds(LCF, L - LCF)
                nc.scalar.activation(deltaAf[:, fs], deltaAf[:, fs],
                                     AF.Ln, bias=1.0)
                nc.scalar.activation(deltaBf[:, fs], deltaBf[:, fs],
                                     AF.Ln, bias=1.0)
                nc.vector.tensor_tensor(dXAf[:, fs], deltaAf[:, fs],
                                        xi2A[:, fs], op=OP.mult)
                nc.vector.tensor_tensor(dXBf[:, fs], deltaBf[:, fs],
                                        xi2B[:, fs], op=OP.mult)
        pP.release()
        for i in range(NCHUNK // 2):
            isl = ts(i, LCF)
            deltaA = deltaAf[:, isl]
            deltaB = deltaBf[:, isl]
            dXA = dXAf[:, isl]
            dXB = dXBf[:, isl]
            brep = brepf[:, isl]
            crep = crepf[:, isl]

            ypsA0 = py.tile([128, LC], f32, name="ypsA0", tag="ypsA0")
            ypsA1 = py.tile([128, LC], f32, name="ypsA1", tag="ypsA1")
            ypsBp = py.tile([128, LC], f32, name="ypsBp", tag="ypsBp")
            ypsB0 = ypsBp[0:DB, :]
            ypsB1 = ypsBp[DB:128, :]
            for j in range(NT):
                jj = j if j < 16 else j - 16
                if j < 16:
                    dsl, xsl = deltaA, dXA
                    p01s = p01all[:, ts(jj, 128)]
                else:
                    dsl, xsl = deltaB, dXB
                    p01s = p01ball[:, ts(jj, 128)]
                dA_t = pT.tile([128, LCF], bf16, name="dA_t")
                dBu = pT.tile([128, LCF], bf16, name="dBu")
                dxc = pT.tile([128, LCF], bf16, name="dxc", bufs=2)
                for h in range(2):
                    drep = ttile(128, LC, "drep")
                    nc.tensor.matmul(drep[:], p01s, dsl[:, ts(h, LC)],
                                     start=True, stop=True)
                    nc.scalar.activation(dA_t[:, ts(h, LC)], drep[:], AF.Exp,
                                         scale=acol[:, j:j + 1])
                    dxrep = ttile(128, LC, "dxrep")
                    nc.tensor.matmul(dxrep[:], p01s, xsl[:, ts(h, LC)],
                                     start=True, stop=True)
                    nc.scalar.activation(dxc[:, ts(h, LC)], dxrep[:], AF.Copy)
                nc.vector.tensor_tensor(dBu[:], dxc[:], brep[:], op=OP.mult)
                h_t = pT.tile([128, LCF], bf16, name="h_t")
                init = 0.0 if i == 0 else hlast[:, j:j + 1]
                nc.vector.tensor_tensor_scan(
                    h_t[:, 0:LC], dA_t[:, 0:LC], dBu[:, 0:LC], init,
                    op0=OP.mult, op1=OP.add)
                nc.vector.tensor_tensor_scan(
                    h_t[:, LC:LCF], dA_t[:, LC:LCF], dBu[:, LC:LCF],
                    h_t[:, LC - 1:LC], op0=OP.mult, op1=OP.add)
                nc.scalar.activation(hlast[:, j:j + 1], h_t[:, LCF - 1:LCF],
                                     AF.Copy)
                ch = pT.tile([128, LCF], bf16, name="ch")
                nc.vector.tensor_tensor(ch[:], h_t[:], crep[:], op=OP.mult)
                if j < 16:
                    nc.tensor.matmul(ypsA0[:], r01all[:, ts(jj, 128)],
                                     ch[:, 0:LC], start=(j == 0), stop=(j == 15))
                    nc.tensor.matmul(ypsA1[:], r01all[:, ts(jj, 128)],
                                     ch[:, LC:LCF], start=(j == 0), stop=(j == 15))
                else:
                    nc.tensor.matmul(ypsB0, r01ball[:, ts(jj, 64)],
                                     ch[:, 0:LC], start=(j == 16), stop=(j == 23),
                                     skip_group_check=True)
                    nc.tensor.matmul(ypsB1, r01ball[:, ts(jj, 64)],
                                     ch[:, LC:LCF], start=(j == 16), stop=(j == 23),
                                     skip_group_check=True)
            yA = pT.tile([DA, LCF], bf16, name="yA", bufs=1)
            yB = pT.tile([DB, LCF], bf16, name="yB", bufs=1)
            for h, (ya_ps, yb_ps) in enumerate([(ypsA0[:], ypsB0), (ypsA1[:], ypsB1)]):
                off = i * LCF + h * LC
                sl = bass.ds(off, LC)
                nc.vector.scalar_tensor_tensor(
                    yA[:, ts(h, LC)], xi2A[:, sl], dpA[:], ya_ps,
                    op0=OP.mult, op1=OP.add)
                nc.vector.scalar_tensor_tensor(
                    yB[:, ts(h, LC)], xi2B[:, sl], dpB[:], yb_ps,
                    op0=OP.mult, op1=OP.add)
            nc.vector.tensor_tensor(yA[:], yA[:], szA[:, ts(i, LCF)], op=OP.mult)
            nc.vector.tensor_tensor(yB[:], yB[:], szB[:, ts(i, LCF)], op=OP.mult)
            mout = pT.tile([C, LCF], bf16, name="mout", bufs=1)
            for h in range(2):
                mps = ttile(C, LC, "mps")
                nc.tensor.matmul(mps[:], outwTA[:], yA[:, ts(h, LC)],
                                 start=True, stop=False)
                nc.tensor.matmul(mps[:], outwTB[:], yB[:, ts(h, LC)],
                                 start=False, stop=True)
                nc.scalar.activation(mout[:, ts(h, LC)], mps[:], AF.Copy)
            # transpose mout back to [wq, c] slices and add DCT residual
            for g in range(2):
                rps = ttile(WQ, 8 * C, "rps", bf16)
                for k in range(8):
                    r = 8 * g + k
                    nc.tensor.matmul(rps[:, ts(k, C)], mout[:, ts(r, WQ)],
                                     identb[:], is_transpose=True,
                                     start=True, stop=True)
                for k in range(8):
                    hq = 16 * i + 8 * g + k
                    nc.vector.tensor_tensor(xdq3[:, :, hq], xdq3[:, :, hq],
                                            rps[:, ts(k, C)], op=OP.add)
        pT.release()
        pG.release()

        # =============== Phase G: IDCT contribution (f16) ===============
        pH = tc.alloc_tile_pool(name="pH", bufs=1)
        # t7[hq, c, W] = sum_wq Z[wq, c, hq] * Mwq[wq, W]; 4 c per psum group
        t7 = pH.tile([HQ, C * W], f16)
        t7_3 = t7.rearrange("p (c w) -> p c w", c=C)
        for g in range(24):
            t7ps = ttile(HQ, 4 * W, "t7ps")
            t7ps3 = t7ps.rearrange("p (c w) -> p c w", c=4)
            for cc in range(4):
                c = 4 * g + cc
                nc.tensor.matmul(t7ps3[:, cc, :], xdq3[:, c, :], mwq[:],
                                 start=True, stop=True)
            nc.scalar.activation(t7_3[:, ts(g, 4), :], t7ps3[:, :, :], AF.Copy)
        ctr_h = contrib.rearrange("c h w -> h c w")
        pSo = tc.alloc_tile_pool(name="pSo", bufs=3)
        for i in range(24):
            cps = ttile(H, LC, "cps")
            nc.tensor.matmul(cps[:], mhq[:], t7[:, ts(i, LC)], start=True, stop=True)
            csb = pSo.tile([H, LC], f32, name="csb")
            nc.scalar.activation(csb[:], cps[:], AF.Copy)
            nc.sync.dma_start(
                ctr_h[:, ts(i, 4), :],
                csb[:].rearrange("h (c w) -> h c w", c=4))
        pSo.release()
        pH.release()
        pD_.release()
        py.release()
        pr.release()
        consts.release()

    nc.compile()
    return nc


def _pad_xpw(xpwT):
    out = np.zeros((D, 96), np.float32)
    out[:, 0:RK] = xpwT[:, 0:RK]
    out[:, 32:32 + S] = xpwT[:, RK:RK + S]
    out[:, 64:64 + S] = xpwT[:, RK + S:RK + 2 * S]
    return out


def _host_inputs(inputs):
    """Build the 8 per-core input maps."""
    import ml_dtypes
    x = inputs["x"]
    ln_w, ln_b = inputs["ln_w"], inputs["ln_b"]
    Mh = _dct_mat(H)
    Mw = _dct_mat(W)
    s01 = np.zeros((S, 128), np.float32)
    p01all = np.zeros((128, 128 * 16), np.float32)
    p01ball = np.zeros((64, 128 * 8), np.float32)
    r01all = np.zeros((128, 128 * 16), np.float32)
    r01ball = np.zeros((128, 64 * 8), np.float32)
    for p in range(128):
        s01[p % S, p] = 1.0
        for j in range(16):
            p01all[8 * j + p // S, 128 * j + p] = 1.0
            r01all[p, 128 * j + 8 * j + p // S] = 1.0
        for j in range(8):
            if 8 * j + p // S < 64:
                p01ball[8 * j + p // S, 128 * j + p] = 1.0
            r01ball[p, 64 * j + 8 * j + p // S] = 1.0
    in_maps = []
    for k in range(8):
        b, q = k // 4, k % 4
        h0 = (q // 2) * HQ
        w0 = (q % 2) * WQ
        in_w2 = (inputs["in_w"][q] * ln_w[None, :]).astype(np.float32)
        bias_e = (inputs["in_w"][q] @ ln_b).astype(np.float32)
        A = (-np.exp(inputs["A_log"][q])).astype(np.float32)  # [D, S]
        acol = np.zeros((128, NT), np.float32)
        for j in range(NT):
            for p in range(128):
                acol[p, j] = A[j * 8 + p // S, p % S]

        m = {
            "xb": x[b],
            "mhqT": Mh[h0:h0 + HQ, :].T,
            "mwqT": Mw[w0:w0 + WQ, :].T,
            "mhq": Mh[h0:h0 + HQ, :],
            "mwq": Mw[w0:w0 + WQ, :],
            "identh": np.eye(WQ, dtype=np.float32),
            "identb": np.eye(C, dtype=np.float32),
            "inwT": in_w2.T,
            "biasi": bias_e[:D, None],
            "biasz": bias_e[D:, None],
            "convw": inputs["conv_w"][q],
            "convb": inputs["conv_b"][q][:, None],
            "xpwT": _pad_xpw(inputs["xp_w"][q].T),
            "dtwT": inputs["dt_w"][q].T,
            "dtb": inputs["dt_b"][q][:, None],
            "acol": acol,
            "dp": inputs["Dp"][q][:, None],
            "outwT": inputs["out_w"][q].T,
            "p01all": p01all,
            "p01ball": p01ball,
            "s01": s01,
            "r01all": r01all,
            "r01ball": r01ball,
        }
        bf = ["inwT", "xpwT", "dtwT", "outwT", "p01all", "p01ball", "s01",
              "r01all", "r01ball", "identb"]
        hf = ["mhqT", "mwqT", "mhq", "mwq", "identh"]
        out = {}
        for kk, vv in m.items():
            if kk in bf:
                dt = ml_dtypes.bfloat16
            elif kk in hf:
                dt = np.float16
            else:
                dt = np.float32
            out[kk] = np.ascontiguousarray(np.asarray(vv, dt))
        in_maps.append(out)
    return in_maps


def kernel(**inputs):
    from concourse import bass_utils
    inputs = {k: np.asarray(v) for k, v in inputs.items()}
    if "nc" not in _BUILT:
        _BUILT["nc"] = _build_nc()
    nc = _BUILT["nc"]
    in_maps = _host_inputs(inputs)
    res = bass_utils.run_bass_kernel_spmd(nc, in_maps, core_ids=list(range(8)))
    out = np.zeros((B, C, H, W), np.float32)
    for k in range(8):
        out[k // 4] += res.results[k]["contrib"]
    return out


if __name__ == "__main__":
    rng = np.random.default_rng(0)
    demo = {
        "x": rng.standard_normal((B, C, H, W)).astype(np.float32),
        "ln_w": np.ones(C, np.float32), "ln_b": np.zeros(C, np.float32),
        "in_w": (rng.standard_normal((4, 2 * D, C)) * 0.02).astype(np.float32),
        "conv_w": (rng.standard_normal((4, D, KCONV)) * 0.02).astype(np.float32),
        "conv_b": np.zeros((4, D), np.float32),
        "xp_w": (rng.standard_normal((4, RK + 2 * S, D)) * 0.02).astype(np.float32),
        "dt_w": (rng.standard_normal((4, D, RK)) * 0.02).astype(np.float32),
        "dt_b": np.full((4, D), -4.0, np.float32),
        "A_log": np.tile(np.log(np.arange(1, S + 1, dtype=np.float32)), (4, D, 1)),
        "Dp": np.ones((4, D), np.float32),
        "out_w": (rng.standard_normal((4, C, D)) * 0.02).astype(np.float32),
    }
    out = kernel(**demo)
    print("kernel output:", out.shape, out.dtype)
